# revision 9
# baseline (speedup 1.0000x reference)
"""Pairwise L2 distance kernel: x [4096,768], anchors [100,64,768] -> [4096,100,64].

Distributed over 8 TRN2 NeuronCores as a 2x4 grid: batch (4096) split in 2,
anchor index (6400) split in 4.  Each core computes a [2048,1600] output block
as sqrt(x2[b] + a2[j] - 2*x@A^T).

The x@A^T matmul runs in fp8e4m3 with DoubleRow (K=256 per pass, fp32 PSUM
accumulate).  Row norms x2: GPSIMD squares each xt DMA slice into fp8 (one big
op per slice), three N=1 DoubleRow ones-matmuls per tile drop the row totals
into a spare PSUM column, ACT copies that out as the Sqrt bias.  Anchor norms
a2: fp8/bf16 squares of at split across ACT (fp8, fast) and DVE (bf16), summed
and broadcast by a ones-matmul (DoubleRow for the fp8 k-pairs, two plain bf16
passes for the DVE pair).  Per-tile epilogue: one DVE add (psum + -0.5*a2 ->
bf16 t) and one ACT sqrt(-2*t + x2) -> bf16 out.  The first two tiles free
their PSUM slot early via a copy and take the a2 add in place once it lands.
Host does layout transforms only (transpose, dtype cast, partition packing).
"""

import sys

import numpy as np

for _p in ("/opt/trn_rl_repo", "/root/.axon_site/_ro/trn_rl_repo"):
    if _p not in sys.path:
        sys.path.append(_p)

import ml_dtypes

import concourse.bass as bass
import concourse.tile as tile
from concourse import bacc, mybir
from concourse.bass import ts
from concourse.bass_utils import run_bass_kernel_spmd

B, C, A, E = 4096, 100, 64, 768
J = C * A                 # 6400 flattened anchors
RB, RJ = 2, 4             # batch groups x anchor groups = 8 cores
MB = B // RB              # 2048 batch rows per core
NJ = J // RJ              # 1600 anchor cols per core
KT = E // 128             # 6 contraction tiles of 128
K2 = KT // 2              # 3 DoubleRow k-pair passes
MT = MB // 128            # 16 m-tiles per core
XT_Q = 8                  # xt arrives in 8 DMA slices (2 m-tiles each)
N_CHUNKS = [(0, 512), (512, 512), (1024, 512), (1536, 64)]
PSW = 2048                # psum tile width (4 banks)
XC = NJ                   # x2 column inside each psum tile
N_WARM = 13               # dummy matmuls spanning the input-DMA head

FP8 = mybir.dt.float8e4
BF16 = mybir.dt.bfloat16
F32 = mybir.dt.float32
NP_FP8 = ml_dtypes.float8_e4m3
NP_BF16 = ml_dtypes.bfloat16


def pack_rows(a2d: np.ndarray) -> np.ndarray:
    """[n*128, F] -> [128, n*F]: row r=k*128+p lands at partition p, block k.
    Makes each SBUF partition's data contiguous in DRAM."""
    n = a2d.shape[0] // 128
    return np.ascontiguousarray(
        a2d.reshape(n, 128, a2d.shape[1]).transpose(1, 0, 2).reshape(128, -1)
    )


def pack_xt(xtg: np.ndarray) -> np.ndarray:
    """[E, MB] -> [128, MT*KT*128] m-major: partition p holds, for each m-tile,
    that tile's KT k-blocks contiguously, so a per-m DMA slice is one fat
    descriptor per partition."""
    return np.ascontiguousarray(
        xtg.reshape(KT, 128, MT, 128).transpose(1, 2, 0, 3).reshape(128, -1)
    )


def build_graph() -> bass.Bass:
    nc = bacc.Bacc(None, target_bir_lowering=False, debug=False, num_devices=8)
    at_ext = nc.declare_dram_parameter("at", [128, KT * NJ], FP8, isOutput=False)
    xt_ext = nc.declare_dram_parameter("xt", [128, MT * KT * 128], FP8, isOutput=False)
    out_ext = nc.declare_dram_parameter("out", [MB, NJ], BF16, isOutput=True)

    with tile.TileContext(nc) as tc:
        with (
            tc.tile_pool(name="big", bufs=1) as big,
            tc.tile_pool(name="atp", bufs=K2) as atp,
            tc.tile_pool(name="xtp", bufs=XT_Q) as xtp,
            tc.tile_pool(name="sxp", bufs=XT_Q) as sxp,
            tc.tile_pool(name="work", bufs=4) as work,
            tc.tile_pool(name="outs", bufs=4) as outs,
            tc.tile_pool(name="psum", bufs=2, space=bass.MemorySpace.PSUM) as psp,
        ):
            # --- constants (DVE memsets at t~0)
            warm_lhs = big.tile([128, 64], BF16, tag="wl")
            nc.vector.memset(warm_lhs, 1.0)
            ones2 = big.tile([128, 2, 1], FP8, tag="o2")
            nc.vector.memset(ones2, 1.0)
            neg2 = big.tile([128, 2, 128], FP8, tag="n2")   # DR -0.5 lhsT
            nc.vector.memset(neg2, -0.5)
            neg1 = big.tile([128, 128], BF16, tag="n1")     # plain -0.5 lhsT
            nc.vector.memset(neg1, -0.5)
            warm_src = big.tile([128, 512], BF16, tag="ws")
            nc.vector.memset(warm_src, 0.125)
            dummy = big.tile([128, 1], BF16, tag="dm")

            # --- tiny prewarm DMA absorbs one-time DGE init latency
            dummy_f8 = big.tile([128, 1], FP8, tag="df")
            # --- input DMAs on sync, priority order: at0, xt0, at1, at2, xt1..7
            at_r = at_ext[:].rearrange("p (q r n) -> p q r n", q=K2, r=2)
            xt_r = xt_ext[:].rearrange("p (s m k c) -> p s m k c", s=XT_Q, m=2, k=KT)
            at_s = [atp.tile([128, 2, NJ], FP8, tag="at", name=f"at{q}")
                    for q in range(K2)]
            xt_s = [xtp.tile([128, 2, KT, 128], FP8, tag="xt", name=f"xt{s}")
                    for s in range(XT_Q)]
            nc.sync.dma_start(out=dummy_f8, in_=at_ext[:, 0:1])
            nc.sync.dma_start(out=at_s[0], in_=at_r[:, 0, :, :])
            nc.sync.dma_start(out=xt_s[0], in_=xt_r[:, 0, :, :, :])
            nc.sync.dma_start(out=at_s[1], in_=at_r[:, 1, :, :])
            nc.sync.dma_start(out=at_s[2], in_=at_r[:, 2, :, :])
            for s in range(1, XT_Q):
                nc.sync.dma_start(out=xt_s[s], in_=xt_r[:, s, :, :, :])

            def xt_sl(m, q):  # lhsT [128, 2, 128] for tile m, k-pair q
                return xt_s[m // 2][:, m % 2, 2 * q : 2 * q + 2, :]

            # --- ACT: load the sqrt table at t~0 (set also contains Square)
            nc.scalar.activation(dummy, warm_src[:, 0:1],
                                 mybir.ActivationFunctionType.Sqrt)

            # --- sq_at: ACT squares the fp8 pairs 0 and 2, DVE pair 1 in bf16
            sq0 = big.tile([128, 2, NJ], FP8, tag="sq0")
            sq1 = big.tile([128, 2, NJ], BF16, tag="sq1")
            sq2 = big.tile([128, 2, NJ], FP8, tag="sq2")
            nc.scalar.activation(sq0, at_s[0],
                                 mybir.ActivationFunctionType.Square)
            nc.scalar.activation(sq2, at_s[2],
                                 mybir.ActivationFunctionType.Square)
            nc.vector.tensor_mul(sq1, at_s[1], at_s[1])

            # --- GPSIMD: one big fp8 square per xt slice (feeds the x2 matmuls)
            sqx_s = []
            for s in range(XT_Q):
                sx = sxp.tile([128, 2, KT, 128], FP8, tag="sx", name=f"sx{s}")
                nc.gpsimd.tensor_mul(sx, xt_s[s], xt_s[s])
                sqx_s.append(sx)

            # --- PE warm-up across the DMA head (p-state ramp)
            warm_ps = psp.tile([128, PSW], F32, tag="ps", name="warm_ps")
            for wi in range(N_WARM):
                nc.tensor.matmul(
                    warm_ps[:64, :512], warm_lhs, warm_src,
                    start=(wi == 0), stop=(wi == N_WARM - 1),
                )

            a2b = big.tile([128, NJ], BF16, tag="a2b")   # -0.5*a2[j] broadcast

            def emit_mains(pts, m):
                for q in range(K2):
                    lhsT = xt_sl(m, q)
                    for n0, w in N_CHUNKS:
                        nc.tensor.matmul(
                            pts[:, n0 : n0 + w], lhsT,
                            at_s[q][:, :, n0 : n0 + w],
                            start=(q == 0), stop=(q == K2 - 1),
                            perf_mode=mybir.MatmulPerfMode.DoubleRow,
                        )

            def emit_x2mm(pts, m):
                sx = sqx_s[m // 2]
                for q in range(K2):
                    nc.tensor.matmul(
                        pts[:, XC : XC + 1],
                        sx[:, m % 2, 2 * q : 2 * q + 2, :], ones2,
                        start=(q == 0), stop=(q == K2 - 1),
                        perf_mode=mybir.MatmulPerfMode.DoubleRow,
                    )

            def emit_a2_setup():
                # psa2 += -0.5 * sum_k at[k,j]^2: DR over the fp8 pairs (0, 2),
                # two plain bf16 passes for pair 1.
                ps = psp.tile([128, PSW], F32, tag="ps", name="psa2")
                for n0, w in N_CHUNKS:
                    nc.tensor.matmul(
                        ps[:, n0 : n0 + w], neg2, sq0[:, :, n0 : n0 + w],
                        start=True, stop=False,
                        perf_mode=mybir.MatmulPerfMode.DoubleRow,
                    )
                for r in range(2):
                    for n0, w in N_CHUNKS:
                        nc.tensor.matmul(
                            ps[:, n0 : n0 + w], neg1, sq1[:, r, n0 : n0 + w],
                            start=False, stop=False,
                        )
                for n0, w in N_CHUNKS:
                    nc.tensor.matmul(
                        ps[:, n0 : n0 + w], neg2, sq2[:, :, n0 : n0 + w],
                        start=False, stop=True,
                        perf_mode=mybir.MatmulPerfMode.DoubleRow,
                    )
                nc.vector.tensor_copy(a2b, ps[:, :NJ])

            tcopies = []
            for m in range(MT):
                pts = psp.tile([128, PSW], F32, tag="ps", name=f"ps{m}")
                emit_mains(pts, m)

                if m < 2:
                    # free the psum slot early (copy on idle ACT); the a2b add
                    # happens later into a fresh tile once a2b lands.
                    t_e = work.tile([128, NJ], BF16, tag="te", name=f"te{m}",
                                    bufs=2)
                    nc.scalar.copy(t_e, pts[:, :NJ])
                    emit_x2mm(pts, m)
                    x2 = work.tile([128, 1], F32, tag="x2", name=f"x2_{m}",
                                   bufs=4)
                    nc.scalar.copy(x2, pts[:, XC : XC + 1])
                    tcopies.append((t_e, x2))
                    if m == 1:
                        emit_a2_setup()
                        for me, (t_e, x2e) in enumerate(tcopies):
                            t_f = work.tile([128, NJ], BF16, tag="t",
                                            name=f"tf{me}", bufs=2)
                            nc.vector.tensor_add(t_f, t_e, a2b)
                            out_t = outs.tile([128, NJ], BF16, tag="out",
                                              name=f"out{me}")
                            nc.scalar.activation(
                                out_t, t_f, mybir.ActivationFunctionType.Sqrt,
                                bias=x2e, scale=-2.0,
                            )
                            nc.sync.dma_start(
                                out=out_ext[ts(me, 128), :], in_=out_t
                            )
                    continue

                emit_x2mm(pts, m)
                x2 = work.tile([128, 1], F32, tag="x2", name=f"x2_{m}", bufs=4)
                nc.scalar.copy(x2, pts[:, XC : XC + 1])

                out_t = outs.tile([128, NJ], BF16, tag="out", name=f"out{m}")
                halves = [(0, NJ)] if m < MT - 2 else [(0, NJ // 2), (NJ // 2, NJ)]
                for h0, h1 in halves:
                    t = work.tile([128, NJ], BF16, tag="t", name=f"t{m}_{h0}",
                                  bufs=2)
                    nc.vector.tensor_add(
                        t[:, : h1 - h0], pts[:, h0:h1], a2b[:, h0:h1]
                    )
                    nc.scalar.activation(
                        out_t[:, h0:h1], t[:, : h1 - h0],
                        mybir.ActivationFunctionType.Sqrt,
                        bias=x2, scale=-2.0,
                    )
                    nc.sync.dma_start(
                        out=out_ext[ts(m, 128), h0:h1], in_=out_t[:, h0:h1]
                    )

    nc.compile()
    return nc


def make_in_maps(x32: np.ndarray, a32: np.ndarray) -> list[dict[str, np.ndarray]]:
    xt_f8 = x32.T.astype(NP_FP8)           # [E, B]
    at_f8 = a32.T.astype(NP_FP8)           # [E, J]
    in_maps = []
    for c in range(8):
        g, h = c // RJ, c % RJ
        in_maps.append({
            "at": pack_rows(at_f8[:, h * NJ : (h + 1) * NJ]),
            "xt": pack_xt(xt_f8[:, g * MB : (g + 1) * MB]),
        })
    return in_maps


def kernel(x: np.ndarray, anchors: np.ndarray) -> np.ndarray:
    x32 = np.asarray(x, dtype=np.float32)
    a32 = np.asarray(anchors, dtype=np.float32).reshape(J, E)

    nc = build_graph()
    in_maps = make_in_maps(x32, a32)
    results = run_bass_kernel_spmd(nc, in_maps, core_ids=list(range(8))).results

    out = np.empty((B, J), dtype=np.float32)
    for c in range(8):
        g, h = c // RJ, c % RJ
        out[g * MB : (g + 1) * MB, h * NJ : (h + 1) * NJ] = results[c][
            "out"
        ].astype(np.float32)
    return out.reshape(B, C, A)


# revision 10
# speedup vs baseline: 1.0517x; 1.0517x over previous
"""Pairwise L2 distance kernel: x [4096,768], anchors [100,64,768] -> [4096,100,64].

Distributed over 8 TRN2 NeuronCores as a 2x4 grid: batch (4096) split in 2,
anchor index (6400) split in 4.  Each core computes a [2048,1600] output block
as sqrt(x2[b] + a2[j] - 2*x@A^T).

The x@A^T matmul runs in fp8e4m3 with DoubleRow (K=256 per pass, fp32 PSUM
accumulate).  Row norms x2: GPSIMD squares each xt DMA slice into fp8 (one big
op per slice), three N=1 DoubleRow ones-matmuls per tile drop the row totals
into a spare PSUM column, ACT copies that out as the Sqrt bias.  Anchor norms
a2: fp8/bf16 squares of at split across ACT (fp8, fast) and DVE (bf16), summed
and broadcast by a ones-matmul (DoubleRow for the fp8 k-pairs, two plain bf16
passes for the DVE pair).  Per-tile epilogue: one DVE add (psum + -0.5*a2 ->
bf16 t) and one ACT sqrt(-2*t + x2) -> bf16 out.  The first two tiles free
their PSUM slot early via a copy and take the a2 add in place once it lands.
Host does layout transforms only (transpose, dtype cast, partition packing).
"""

import sys

import numpy as np

for _p in ("/opt/trn_rl_repo", "/root/.axon_site/_ro/trn_rl_repo"):
    if _p not in sys.path:
        sys.path.append(_p)

import ml_dtypes

import concourse.bass as bass
import concourse.tile as tile
from concourse import bacc, mybir
from concourse.bass import ts
from concourse.bass_utils import run_bass_kernel_spmd

B, C, A, E = 4096, 100, 64, 768
J = C * A                 # 6400 flattened anchors
RB, RJ = 2, 4             # batch groups x anchor groups = 8 cores
MB = B // RB              # 2048 batch rows per core
NJ = J // RJ              # 1600 anchor cols per core
KT = E // 128             # 6 contraction tiles of 128
K2 = KT // 2              # 3 DoubleRow k-pair passes
MT = MB // 128            # 16 m-tiles per core
XT_Q = 8                  # xt arrives in 8 DMA slices (2 m-tiles each)
N_CHUNKS = [(0, 512), (512, 512), (1024, 512), (1536, 64)]
PSW = 2048                # psum tile width (4 banks)
XC = NJ                   # x2 column inside each psum tile
N_WARM = 13               # dummy matmuls spanning the input-DMA head

FP8 = mybir.dt.float8e4
BF16 = mybir.dt.bfloat16
F32 = mybir.dt.float32
NP_FP8 = ml_dtypes.float8_e4m3
NP_BF16 = ml_dtypes.bfloat16


def pack_rows(a2d: np.ndarray) -> np.ndarray:
    """[n*128, F] -> [128, n*F]: row r=k*128+p lands at partition p, block k.
    Makes each SBUF partition's data contiguous in DRAM."""
    n = a2d.shape[0] // 128
    return np.ascontiguousarray(
        a2d.reshape(n, 128, a2d.shape[1]).transpose(1, 0, 2).reshape(128, -1)
    )


def pack_xt(xtg: np.ndarray) -> np.ndarray:
    """[E, MB] -> [128, MT*KT*128] m-major: partition p holds, for each m-tile,
    that tile's KT k-blocks contiguously, so a per-m DMA slice is one fat
    descriptor per partition."""
    return np.ascontiguousarray(
        xtg.reshape(KT, 128, MT, 128).transpose(1, 2, 0, 3).reshape(128, -1)
    )


def build_graph() -> bass.Bass:
    nc = bacc.Bacc(None, target_bir_lowering=False, debug=False, num_devices=8)
    at_ext = nc.declare_dram_parameter("at", [128, KT * NJ], FP8, isOutput=False)
    xt_ext = nc.declare_dram_parameter("xt", [128, MT * KT * 128], FP8, isOutput=False)
    out_ext = nc.declare_dram_parameter("out", [MB, NJ], BF16, isOutput=True)

    with tile.TileContext(nc) as tc:
        with (
            tc.tile_pool(name="big", bufs=1) as big,
            tc.tile_pool(name="atp", bufs=K2) as atp,
            tc.tile_pool(name="xtp", bufs=XT_Q) as xtp,
            tc.tile_pool(name="sxp", bufs=XT_Q) as sxp,
            tc.tile_pool(name="work", bufs=4) as work,
            tc.tile_pool(name="outs", bufs=4) as outs,
            tc.tile_pool(name="psum", bufs=2, space=bass.MemorySpace.PSUM) as psp,
        ):
            # --- constants (DVE memsets at t~0)
            warm_lhs = big.tile([128, 64], BF16, tag="wl")
            nc.vector.memset(warm_lhs, 1.0)
            ones2 = big.tile([128, 2, 1], FP8, tag="o2")
            nc.vector.memset(ones2, 1.0)
            neg2 = big.tile([128, 2, 128], FP8, tag="n2")   # DR -0.5 lhsT
            nc.vector.memset(neg2, -0.5)
            neg1 = big.tile([128, 128], BF16, tag="n1")     # plain -0.5 lhsT
            nc.vector.memset(neg1, -0.5)
            warm_src = big.tile([128, 512], BF16, tag="ws")
            nc.vector.memset(warm_src, 0.125)
            dummy = big.tile([128, 1], BF16, tag="dm")

            # --- input DMAs on sync, priority order: at0, xt0, at1, at2, xt1..7
            at_r = at_ext[:].rearrange("p (q r n) -> p q r n", q=K2, r=2)
            xt_r = xt_ext[:].rearrange("p (s m k c) -> p s m k c", s=XT_Q, m=2, k=KT)
            at_s = [atp.tile([128, 2, NJ], FP8, tag="at", name=f"at{q}")
                    for q in range(K2)]
            xt_s = [xtp.tile([128, 2, KT, 128], FP8, tag="xt", name=f"xt{s}")
                    for s in range(XT_Q)]
            nc.sync.dma_start(out=at_s[0], in_=at_r[:, 0, :, :])
            nc.sync.dma_start(out=xt_s[0], in_=xt_r[:, 0, :, :, :])
            nc.sync.dma_start(out=at_s[1], in_=at_r[:, 1, :, :])
            nc.sync.dma_start(out=at_s[2], in_=at_r[:, 2, :, :])
            for s in range(1, XT_Q):
                nc.sync.dma_start(out=xt_s[s], in_=xt_r[:, s, :, :, :])

            def xt_sl(m, q):  # lhsT [128, 2, 128] for tile m, k-pair q
                return xt_s[m // 2][:, m % 2, 2 * q : 2 * q + 2, :]

            # --- ACT: load the sqrt table at t~0 (set also contains Square)
            nc.scalar.activation(dummy, warm_src[:, 0:1],
                                 mybir.ActivationFunctionType.Sqrt)

            # --- sq_at: ACT squares the fp8 pairs 0 and 2, DVE pair 1 in bf16
            sq0 = big.tile([128, 2, NJ], FP8, tag="sq0")
            sq1 = big.tile([128, 2, NJ], BF16, tag="sq1")
            sq2 = big.tile([128, 2, NJ], FP8, tag="sq2")
            nc.scalar.activation(sq0, at_s[0],
                                 mybir.ActivationFunctionType.Square)
            nc.scalar.activation(sq2, at_s[2],
                                 mybir.ActivationFunctionType.Square)
            nc.vector.tensor_mul(sq1, at_s[1], at_s[1])

            # --- GPSIMD: one big fp8 square per xt slice (feeds the x2 matmuls)
            sqx_s = []
            for s in range(XT_Q):
                sx = sxp.tile([128, 2, KT, 128], FP8, tag="sx", name=f"sx{s}")
                nc.gpsimd.tensor_mul(sx, xt_s[s], xt_s[s])
                sqx_s.append(sx)

            # --- PE warm-up across the DMA head (p-state ramp)
            warm_ps = psp.tile([128, PSW], F32, tag="ps", name="warm_ps")
            for wi in range(N_WARM):
                nc.tensor.matmul(
                    warm_ps[:64, :512], warm_lhs, warm_src,
                    start=(wi == 0), stop=(wi == N_WARM - 1),
                )

            a2b = big.tile([128, NJ], BF16, tag="a2b")   # -0.5*a2[j] broadcast

            def emit_mains(pts, m):
                for q in range(K2):
                    lhsT = xt_sl(m, q)
                    for n0, w in N_CHUNKS:
                        nc.tensor.matmul(
                            pts[:, n0 : n0 + w], lhsT,
                            at_s[q][:, :, n0 : n0 + w],
                            start=(q == 0), stop=(q == K2 - 1),
                            perf_mode=mybir.MatmulPerfMode.DoubleRow,
                        )

            def emit_x2mm(pts, m):
                sx = sqx_s[m // 2]
                for q in range(K2):
                    nc.tensor.matmul(
                        pts[:, XC : XC + 1],
                        sx[:, m % 2, 2 * q : 2 * q + 2, :], ones2,
                        start=(q == 0), stop=(q == K2 - 1),
                        perf_mode=mybir.MatmulPerfMode.DoubleRow,
                    )

            def emit_a2_setup():
                # psa2 += -0.5 * sum_k at[k,j]^2: DR over the fp8 pairs (0, 2),
                # two plain bf16 passes for pair 1.
                ps = psp.tile([128, PSW], F32, tag="ps", name="psa2")
                for n0, w in N_CHUNKS:
                    nc.tensor.matmul(
                        ps[:, n0 : n0 + w], neg2, sq0[:, :, n0 : n0 + w],
                        start=True, stop=False,
                        perf_mode=mybir.MatmulPerfMode.DoubleRow,
                    )
                for r in range(2):
                    for n0, w in N_CHUNKS:
                        nc.tensor.matmul(
                            ps[:, n0 : n0 + w], neg1, sq1[:, r, n0 : n0 + w],
                            start=False, stop=False,
                        )
                for n0, w in N_CHUNKS:
                    nc.tensor.matmul(
                        ps[:, n0 : n0 + w], neg2, sq2[:, :, n0 : n0 + w],
                        start=False, stop=True,
                        perf_mode=mybir.MatmulPerfMode.DoubleRow,
                    )
                nc.vector.tensor_copy(a2b, ps[:, :NJ])

            tcopies = []
            for m in range(MT):
                pts = psp.tile([128, PSW], F32, tag="ps", name=f"ps{m}")
                emit_mains(pts, m)

                if m < 2:
                    # free the psum slot early (copy on idle ACT); the a2b add
                    # happens later into a fresh tile once a2b lands.
                    t_e = work.tile([128, NJ], BF16, tag="te", name=f"te{m}",
                                    bufs=2)
                    nc.scalar.copy(t_e, pts[:, :NJ])
                    emit_x2mm(pts, m)
                    x2 = work.tile([128, 1], F32, tag="x2", name=f"x2_{m}",
                                   bufs=4)
                    nc.scalar.copy(x2, pts[:, XC : XC + 1])
                    tcopies.append((t_e, x2))
                    if m == 1:
                        emit_a2_setup()
                    continue

                emit_x2mm(pts, m)
                x2 = work.tile([128, 1], F32, tag="x2", name=f"x2_{m}", bufs=4)
                nc.scalar.copy(x2, pts[:, XC : XC + 1])

                out_t = outs.tile([128, NJ], BF16, tag="out", name=f"out{m}")
                halves = [(0, NJ)] if m < MT - 2 else [(0, NJ // 2), (NJ // 2, NJ)]
                for h0, h1 in halves:
                    t = work.tile([128, NJ], BF16, tag="t", name=f"t{m}_{h0}",
                                  bufs=2)
                    nc.vector.tensor_add(
                        t[:, : h1 - h0], pts[:, h0:h1], a2b[:, h0:h1]
                    )
                    nc.scalar.activation(
                        out_t[:, h0:h1], t[:, : h1 - h0],
                        mybir.ActivationFunctionType.Sqrt,
                        bias=x2, scale=-2.0,
                    )
                    nc.sync.dma_start(
                        out=out_ext[ts(m, 128), h0:h1], in_=out_t[:, h0:h1]
                    )
                if m - 2 < len(tcopies):
                    # finish a deferred early tile behind this tile's add so
                    # the DVE burst never delays a psum-freeing add
                    t_e, x2e = tcopies[m - 2]
                    t_f = work.tile([128, NJ], BF16, tag="t",
                                    name=f"tf{m - 2}", bufs=2)
                    nc.vector.tensor_add(t_f, t_e, a2b)
                    out_t = outs.tile([128, NJ], BF16, tag="out",
                                      name=f"oute{m - 2}")
                    nc.scalar.activation(
                        out_t, t_f, mybir.ActivationFunctionType.Sqrt,
                        bias=x2e, scale=-2.0,
                    )
                    nc.sync.dma_start(
                        out=out_ext[ts(m - 2, 128), :], in_=out_t
                    )

    nc.compile()
    return nc


def make_in_maps(x32: np.ndarray, a32: np.ndarray) -> list[dict[str, np.ndarray]]:
    xt_f8 = x32.T.astype(NP_FP8)           # [E, B]
    at_f8 = a32.T.astype(NP_FP8)           # [E, J]
    in_maps = []
    for c in range(8):
        g, h = c // RJ, c % RJ
        in_maps.append({
            "at": pack_rows(at_f8[:, h * NJ : (h + 1) * NJ]),
            "xt": pack_xt(xt_f8[:, g * MB : (g + 1) * MB]),
        })
    return in_maps


def kernel(x: np.ndarray, anchors: np.ndarray) -> np.ndarray:
    x32 = np.asarray(x, dtype=np.float32)
    a32 = np.asarray(anchors, dtype=np.float32).reshape(J, E)

    nc = build_graph()
    in_maps = make_in_maps(x32, a32)
    results = run_bass_kernel_spmd(nc, in_maps, core_ids=list(range(8))).results

    out = np.empty((B, J), dtype=np.float32)
    for c in range(8):
        g, h = c // RJ, c % RJ
        out[g * MB : (g + 1) * MB, h * NJ : (h + 1) * NJ] = results[c][
            "out"
        ].astype(np.float32)
    return out.reshape(B, C, A)


# revision 11
# speedup vs baseline: 1.0530x; 1.0012x over previous
"""Pairwise L2 distance kernel: x [4096,768], anchors [100,64,768] -> [4096,100,64].

Distributed over 8 TRN2 NeuronCores as a 2x4 grid: batch (4096) split in 2,
anchor index (6400) split in 4.  Each core computes a [2048,1600] output block
as sqrt(x2[b] + a2[j] - 2*x@A^T).

The x@A^T matmul runs in fp8e4m3 with DoubleRow (K=256 per pass, fp32 PSUM
accumulate).  Row norms x2: GPSIMD squares each xt DMA slice into fp8 (one big
op per slice), three N=1 DoubleRow ones-matmuls per tile drop the row totals
into a spare PSUM column, ACT copies that out as the Sqrt bias.  Anchor norms
a2: fp8/bf16 squares of at split across ACT (fp8, fast) and DVE (bf16), summed
and broadcast by a ones-matmul (DoubleRow for the fp8 k-pairs, two plain bf16
passes for the DVE pair).  Per-tile epilogue: one DVE add (psum + -0.5*a2 ->
bf16 t) and one ACT sqrt(-2*t + x2) -> bf16 out.  The first two tiles free
their PSUM slot early via a copy and take the a2 add in place once it lands.
Host does layout transforms only (transpose, dtype cast, partition packing).
"""

import sys

import numpy as np

for _p in ("/opt/trn_rl_repo", "/root/.axon_site/_ro/trn_rl_repo"):
    if _p not in sys.path:
        sys.path.append(_p)

import ml_dtypes

import concourse.bass as bass
import concourse.tile as tile
from concourse import bacc, mybir
from concourse.bass import ts
from concourse.bass_utils import run_bass_kernel_spmd

B, C, A, E = 4096, 100, 64, 768
J = C * A                 # 6400 flattened anchors
RB, RJ = 2, 4             # batch groups x anchor groups = 8 cores
MB = B // RB              # 2048 batch rows per core
NJ = J // RJ              # 1600 anchor cols per core
KT = E // 128             # 6 contraction tiles of 128
K2 = KT // 2              # 3 DoubleRow k-pair passes
MT = MB // 128            # 16 m-tiles per core
XT_Q = 8                  # xt arrives in 8 DMA slices (2 m-tiles each)
N_CHUNKS = [(0, 512), (512, 512), (1024, 512), (1536, 64)]
PSW = 2048                # psum tile width (4 banks)
XC = NJ                   # x2 column inside each psum tile
N_WARM = 13               # dummy matmuls spanning the input-DMA head

FP8 = mybir.dt.float8e4
BF16 = mybir.dt.bfloat16
F32 = mybir.dt.float32
NP_FP8 = ml_dtypes.float8_e4m3
NP_BF16 = ml_dtypes.bfloat16


def pack_rows(a2d: np.ndarray) -> np.ndarray:
    """[n*128, F] -> [128, n*F]: row r=k*128+p lands at partition p, block k.
    Makes each SBUF partition's data contiguous in DRAM."""
    n = a2d.shape[0] // 128
    return np.ascontiguousarray(
        a2d.reshape(n, 128, a2d.shape[1]).transpose(1, 0, 2).reshape(128, -1)
    )


def pack_xt(xtg: np.ndarray) -> np.ndarray:
    """[E, MB] -> [128, MT*KT*128] m-major: partition p holds, for each m-tile,
    that tile's KT k-blocks contiguously, so a per-m DMA slice is one fat
    descriptor per partition."""
    return np.ascontiguousarray(
        xtg.reshape(KT, 128, MT, 128).transpose(1, 2, 0, 3).reshape(128, -1)
    )


def build_graph() -> bass.Bass:
    nc = bacc.Bacc(None, target_bir_lowering=False, debug=False, num_devices=8)
    at_ext = nc.declare_dram_parameter("at", [128, KT * NJ], FP8, isOutput=False)
    xt_ext = nc.declare_dram_parameter("xt", [128, MT * KT * 128], FP8, isOutput=False)
    out_ext = nc.declare_dram_parameter("out", [MB, NJ], BF16, isOutput=True)

    with tile.TileContext(nc) as tc:
        with (
            tc.tile_pool(name="big", bufs=1) as big,
            tc.tile_pool(name="atp", bufs=K2) as atp,
            tc.tile_pool(name="xtp", bufs=XT_Q) as xtp,
            tc.tile_pool(name="sxp", bufs=XT_Q) as sxp,
            tc.tile_pool(name="work", bufs=4) as work,
            tc.tile_pool(name="outs", bufs=4) as outs,
            tc.tile_pool(name="psum", bufs=2, space=bass.MemorySpace.PSUM) as psp,
        ):
            # --- constants (DVE memsets at t~0)
            warm_lhs = big.tile([128, 64], BF16, tag="wl")
            nc.vector.memset(warm_lhs, 1.0)
            ones2 = big.tile([128, 2, 1], FP8, tag="o2")
            nc.vector.memset(ones2, 1.0)
            neg2 = big.tile([128, 2, 128], FP8, tag="n2")   # DR -0.5 lhsT
            nc.vector.memset(neg2, -0.5)
            neg1 = big.tile([128, 128], BF16, tag="n1")     # plain -0.5 lhsT
            nc.vector.memset(neg1, -0.5)
            warm_src = big.tile([128, 512], BF16, tag="ws")
            nc.vector.memset(warm_src, 0.125)
            dummy = big.tile([128, 1], BF16, tag="dm")

            # --- input DMAs on sync, priority order: at0, xt0, at1, at2, xt1..7
            at_r = at_ext[:].rearrange("p (q r n) -> p q r n", q=K2, r=2)
            xt_r = xt_ext[:].rearrange("p (s m k c) -> p s m k c", s=XT_Q, m=2, k=KT)
            at_s = [atp.tile([128, 2, NJ], FP8, tag="at", name=f"at{q}")
                    for q in range(K2)]
            xt_s = [xtp.tile([128, 2, KT, 128], FP8, tag="xt", name=f"xt{s}")
                    for s in range(XT_Q)]
            nc.sync.dma_start(out=at_s[0], in_=at_r[:, 0, :, :])
            nc.sync.dma_start(out=xt_s[0], in_=xt_r[:, 0, :, :, :])
            nc.sync.dma_start(out=at_s[1], in_=at_r[:, 1, :, :])
            nc.sync.dma_start(out=at_s[2], in_=at_r[:, 2, :, :])
            for s in range(1, XT_Q):
                nc.sync.dma_start(out=xt_s[s], in_=xt_r[:, s, :, :, :])

            def xt_sl(m, q):  # lhsT [128, 2, 128] for tile m, k-pair q
                return xt_s[m // 2][:, m % 2, 2 * q : 2 * q + 2, :]

            # --- ACT: load the sqrt table at t~0 (set also contains Square)
            nc.scalar.activation(dummy, warm_src[:, 0:1],
                                 mybir.ActivationFunctionType.Sqrt)

            # --- sq_at: ACT squares the fp8 pairs 0 and 2, DVE pair 1 in bf16
            sq0 = big.tile([128, 2, NJ], FP8, tag="sq0")
            sq1 = big.tile([128, 2, NJ], BF16, tag="sq1")
            sq2 = big.tile([128, 2, NJ], FP8, tag="sq2")
            nc.scalar.activation(sq0, at_s[0],
                                 mybir.ActivationFunctionType.Square)
            nc.scalar.activation(sq2[:, :, : NJ // 2], at_s[2][:, :, : NJ // 2],
                                 mybir.ActivationFunctionType.Square)
            nc.vector.tensor_mul(sq1, at_s[1], at_s[1])

            # --- GPSIMD: sqxt0 first (x2mm0 needs it), then sq2's other half,
            # then the remaining xt squares.
            sqx_s = [sxp.tile([128, 2, KT, 128], FP8, tag="sx", name=f"sx{s}")
                     for s in range(XT_Q)]
            nc.gpsimd.tensor_mul(sqx_s[0], xt_s[0], xt_s[0])
            nc.gpsimd.tensor_mul(sq2[:, :, NJ // 2 :], at_s[2][:, :, NJ // 2 :],
                                 at_s[2][:, :, NJ // 2 :])
            for s in range(1, XT_Q):
                nc.gpsimd.tensor_mul(sqx_s[s], xt_s[s], xt_s[s])

            # --- PE warm-up across the DMA head (p-state ramp)
            warm_ps = psp.tile([128, PSW], F32, tag="ps", name="warm_ps")
            for wi in range(N_WARM):
                nc.tensor.matmul(
                    warm_ps[:64, :512], warm_lhs, warm_src,
                    start=(wi == 0), stop=(wi == N_WARM - 1),
                )

            a2b = big.tile([128, NJ], BF16, tag="a2b")   # -0.5*a2[j] broadcast

            def emit_mains(pts, m):
                for q in range(K2):
                    lhsT = xt_sl(m, q)
                    for n0, w in N_CHUNKS:
                        nc.tensor.matmul(
                            pts[:, n0 : n0 + w], lhsT,
                            at_s[q][:, :, n0 : n0 + w],
                            start=(q == 0), stop=(q == K2 - 1),
                            perf_mode=mybir.MatmulPerfMode.DoubleRow,
                        )

            def emit_x2mm(pts, m):
                sx = sqx_s[m // 2]
                for q in range(K2):
                    nc.tensor.matmul(
                        pts[:, XC : XC + 1],
                        sx[:, m % 2, 2 * q : 2 * q + 2, :], ones2,
                        start=(q == 0), stop=(q == K2 - 1),
                        perf_mode=mybir.MatmulPerfMode.DoubleRow,
                    )

            def emit_a2_pass(ps, q):
                # psa2 += -0.5 * at[k-pair q]^2: DR for the fp8 pairs (0, 2),
                # two plain bf16 passes for pair 1.
                if q == 0:
                    for n0, w in N_CHUNKS:
                        nc.tensor.matmul(
                            ps[:, n0 : n0 + w], neg2, sq0[:, :, n0 : n0 + w],
                            start=True, stop=False,
                            perf_mode=mybir.MatmulPerfMode.DoubleRow,
                        )
                elif q == 1:
                    for r in range(2):
                        for n0, w in N_CHUNKS:
                            nc.tensor.matmul(
                                ps[:, n0 : n0 + w], neg1,
                                sq1[:, r, n0 : n0 + w],
                                start=False, stop=False,
                            )
                else:
                    for n0, w in N_CHUNKS:
                        nc.tensor.matmul(
                            ps[:, n0 : n0 + w], neg2, sq2[:, :, n0 : n0 + w],
                            start=False, stop=True,
                            perf_mode=mybir.MatmulPerfMode.DoubleRow,
                        )

            tcopies = []
            for m in range(MT):
                pts = psp.tile([128, PSW], F32, tag="ps", name=f"ps{m}")
                emit_mains(pts, m)

                if m < 2:
                    # free the psum slot early (copy on idle ACT); the a2b add
                    # happens later into a fresh tile once a2b lands.
                    t_e = work.tile([128, NJ], BF16, tag="te", name=f"te{m}",
                                    bufs=2)
                    nc.scalar.copy(t_e, pts[:, :NJ])
                    emit_x2mm(pts, m)
                    x2 = work.tile([128, 1], F32, tag="x2", name=f"x2_{m}",
                                   bufs=4)
                    nc.scalar.copy(x2, pts[:, XC : XC + 1])
                    tcopies.append((t_e, x2))
                    if m == 0:
                        psa2 = psp.tile([128, PSW], F32, tag="ps", name="psa2")
                        emit_a2_pass(psa2, 0)
                    else:
                        emit_a2_pass(psa2, 1)
                        emit_a2_pass(psa2, 2)
                        nc.vector.tensor_copy(a2b, psa2[:, :NJ])
                    continue

                emit_x2mm(pts, m)
                x2 = work.tile([128, 1], F32, tag="x2", name=f"x2_{m}", bufs=4)
                nc.scalar.copy(x2, pts[:, XC : XC + 1])

                out_t = outs.tile([128, NJ], BF16, tag="out", name=f"out{m}")
                halves = [(0, NJ)] if m < MT - 2 else [(0, NJ // 2), (NJ // 2, NJ)]
                for h0, h1 in halves:
                    t = work.tile([128, NJ], BF16, tag="t", name=f"t{m}_{h0}",
                                  bufs=2)
                    nc.vector.tensor_add(
                        t[:, : h1 - h0], pts[:, h0:h1], a2b[:, h0:h1]
                    )
                    nc.scalar.activation(
                        out_t[:, h0:h1], t[:, : h1 - h0],
                        mybir.ActivationFunctionType.Sqrt,
                        bias=x2, scale=-2.0,
                    )
                    nc.sync.dma_start(
                        out=out_ext[ts(m, 128), h0:h1], in_=out_t[:, h0:h1]
                    )
                if m - 2 < len(tcopies):
                    # finish a deferred early tile behind this tile's add so
                    # the DVE burst never delays a psum-freeing add
                    t_e, x2e = tcopies[m - 2]
                    t_f = work.tile([128, NJ], BF16, tag="t",
                                    name=f"tf{m - 2}", bufs=2)
                    nc.vector.tensor_add(t_f, t_e, a2b)
                    out_t = outs.tile([128, NJ], BF16, tag="out",
                                      name=f"oute{m - 2}")
                    nc.scalar.activation(
                        out_t, t_f, mybir.ActivationFunctionType.Sqrt,
                        bias=x2e, scale=-2.0,
                    )
                    nc.sync.dma_start(
                        out=out_ext[ts(m - 2, 128), :], in_=out_t
                    )

    nc.compile()
    return nc


def make_in_maps(x32: np.ndarray, a32: np.ndarray) -> list[dict[str, np.ndarray]]:
    xt_f8 = x32.T.astype(NP_FP8)           # [E, B]
    at_f8 = a32.T.astype(NP_FP8)           # [E, J]
    in_maps = []
    for c in range(8):
        g, h = c // RJ, c % RJ
        in_maps.append({
            "at": pack_rows(at_f8[:, h * NJ : (h + 1) * NJ]),
            "xt": pack_xt(xt_f8[:, g * MB : (g + 1) * MB]),
        })
    return in_maps


def kernel(x: np.ndarray, anchors: np.ndarray) -> np.ndarray:
    x32 = np.asarray(x, dtype=np.float32)
    a32 = np.asarray(anchors, dtype=np.float32).reshape(J, E)

    nc = build_graph()
    in_maps = make_in_maps(x32, a32)
    results = run_bass_kernel_spmd(nc, in_maps, core_ids=list(range(8))).results

    out = np.empty((B, J), dtype=np.float32)
    for c in range(8):
        g, h = c // RJ, c % RJ
        out[g * MB : (g + 1) * MB, h * NJ : (h + 1) * NJ] = results[c][
            "out"
        ].astype(np.float32)
    return out.reshape(B, C, A)


# revision 12
# speedup vs baseline: 1.0548x; 1.0017x over previous
"""Pairwise L2 distance kernel: x [4096,768], anchors [100,64,768] -> [4096,100,64].

Distributed over 8 TRN2 NeuronCores as a 2x4 grid: batch (4096) split in 2,
anchor index (6400) split in 4.  Each core computes a [2048,1600] output block
as sqrt(x2[b] + a2[j] - 2*x@A^T).

The x@A^T matmul runs in fp8e4m3 with DoubleRow (K=256 per pass, fp32 PSUM
accumulate).  Row norms x2: GPSIMD squares each xt DMA slice into fp8 (one big
op per slice), three N=1 DoubleRow ones-matmuls per tile drop the row totals
into a spare PSUM column, ACT copies that out as the Sqrt bias.  Anchor norms
a2: fp8/bf16 squares of at split across ACT (fp8, fast) and DVE (bf16), summed
and broadcast by a ones-matmul (DoubleRow for the fp8 k-pairs, two plain bf16
passes for the DVE pair).  Per-tile epilogue: one DVE add (psum + -0.5*a2 ->
bf16 t) and one ACT sqrt(-2*t + x2) -> bf16 out.  The first two tiles free
their PSUM slot early via a copy and take the a2 add in place once it lands.
Host does layout transforms only (transpose, dtype cast, partition packing).
"""

import sys

import numpy as np

for _p in ("/opt/trn_rl_repo", "/root/.axon_site/_ro/trn_rl_repo"):
    if _p not in sys.path:
        sys.path.append(_p)

import ml_dtypes

import concourse.bass as bass
import concourse.tile as tile
from concourse import bacc, mybir
from concourse.bass import ts
from concourse.bass_utils import run_bass_kernel_spmd

B, C, A, E = 4096, 100, 64, 768
J = C * A                 # 6400 flattened anchors
RB, RJ = 2, 4             # batch groups x anchor groups = 8 cores
MB = B // RB              # 2048 batch rows per core
NJ = J // RJ              # 1600 anchor cols per core
KT = E // 128             # 6 contraction tiles of 128
K2 = KT // 2              # 3 DoubleRow k-pair passes
MT = MB // 128            # 16 m-tiles per core
XT_Q = 8                  # xt arrives in 8 DMA slices (2 m-tiles each)
N_CHUNKS = [(0, 512), (512, 512), (1024, 512), (1536, 64)]
PSW = 2048                # psum tile width (4 banks)
XC = NJ                   # x2 column inside each psum tile
N_WARM = 13               # dummy matmuls spanning the input-DMA head

FP8 = mybir.dt.float8e4
BF16 = mybir.dt.bfloat16
F32 = mybir.dt.float32
NP_FP8 = ml_dtypes.float8_e4m3
NP_BF16 = ml_dtypes.bfloat16


def pack_rows(a2d: np.ndarray) -> np.ndarray:
    """[n*128, F] -> [128, n*F]: row r=k*128+p lands at partition p, block k.
    Makes each SBUF partition's data contiguous in DRAM."""
    n = a2d.shape[0] // 128
    return np.ascontiguousarray(
        a2d.reshape(n, 128, a2d.shape[1]).transpose(1, 0, 2).reshape(128, -1)
    )


def pack_xt(xtg: np.ndarray) -> np.ndarray:
    """[E, MB] -> [128, MT*KT*128] m-major: partition p holds, for each m-tile,
    that tile's KT k-blocks contiguously, so a per-m DMA slice is one fat
    descriptor per partition."""
    return np.ascontiguousarray(
        xtg.reshape(KT, 128, MT, 128).transpose(1, 2, 0, 3).reshape(128, -1)
    )


def build_graph() -> bass.Bass:
    nc = bacc.Bacc(None, target_bir_lowering=False, debug=False, num_devices=8)
    at_ext = nc.declare_dram_parameter("at", [128, KT * NJ], FP8, isOutput=False)
    xt_ext = nc.declare_dram_parameter("xt", [128, MT * KT * 128], FP8, isOutput=False)
    out_ext = nc.declare_dram_parameter("out", [MB, NJ], BF16, isOutput=True)

    with tile.TileContext(nc) as tc:
        with (
            tc.tile_pool(name="big", bufs=1) as big,
            tc.tile_pool(name="atp", bufs=K2) as atp,
            tc.tile_pool(name="xtp", bufs=XT_Q) as xtp,
            tc.tile_pool(name="sxp", bufs=XT_Q) as sxp,
            tc.tile_pool(name="work", bufs=4) as work,
            tc.tile_pool(name="outs", bufs=4) as outs,
            tc.tile_pool(name="psum", bufs=2, space=bass.MemorySpace.PSUM) as psp,
        ):
            # --- constants (DVE memsets at t~0)
            warm_lhs = big.tile([128, 64], BF16, tag="wl")
            nc.vector.memset(warm_lhs, 1.0)
            ones2 = big.tile([128, 2, 1], FP8, tag="o2")
            nc.vector.memset(ones2, 1.0)
            neg2 = big.tile([128, 2, 128], FP8, tag="n2")   # DR -0.5 lhsT
            nc.vector.memset(neg2, -0.5)
            neg1 = big.tile([128, 128], BF16, tag="n1")     # plain -0.5 lhsT
            nc.vector.memset(neg1, -0.5)
            warm_src = big.tile([128, 512], BF16, tag="ws")
            nc.vector.memset(warm_src, 0.125)
            dummy = big.tile([128, 1], BF16, tag="dm")

            # --- input DMAs on sync, priority order: at0, xt0, at1, at2, xt1..7
            at_r = at_ext[:].rearrange("p (q r n) -> p q r n", q=K2, r=2)
            xt_r = xt_ext[:].rearrange("p (s m k c) -> p s m k c", s=XT_Q, m=2, k=KT)
            at_s = [atp.tile([128, 2, NJ], FP8, tag="at", name=f"at{q}")
                    for q in range(K2)]
            xt_s = [xtp.tile([128, 2, KT, 128], FP8, tag="xt", name=f"xt{s}")
                    for s in range(XT_Q)]
            nc.sync.dma_start(out=at_s[0], in_=at_r[:, 0, :, :])
            nc.sync.dma_start(out=xt_s[0], in_=xt_r[:, 0, :, :, :])
            nc.sync.dma_start(out=at_s[1], in_=at_r[:, 1, :, :])
            nc.sync.dma_start(out=at_s[2], in_=at_r[:, 2, :, :])
            for s in range(1, XT_Q):
                nc.sync.dma_start(out=xt_s[s], in_=xt_r[:, s, :, :, :])

            def xt_sl(m, q):  # lhsT [128, 2, 128] for tile m, k-pair q
                return xt_s[m // 2][:, m % 2, 2 * q : 2 * q + 2, :]

            # --- ACT: load the sqrt table at t~0 (set also contains Square)
            nc.scalar.activation(dummy, warm_src[:, 0:1],
                                 mybir.ActivationFunctionType.Sqrt)

            # --- sq_at: ACT squares the fp8 pairs 0 and 2, DVE pair 1 in bf16
            sq0 = big.tile([128, 2, NJ], FP8, tag="sq0")
            sq1 = big.tile([128, 2, NJ], BF16, tag="sq1")
            sq2 = big.tile([128, 2, NJ], FP8, tag="sq2")
            nc.scalar.activation(sq0, at_s[0],
                                 mybir.ActivationFunctionType.Square)
            nc.scalar.activation(sq2[:, 1, :], at_s[2][:, 1, :],
                                 mybir.ActivationFunctionType.Square)
            nc.vector.tensor_mul(sq1, at_s[1], at_s[1])

            # --- GPSIMD: sqxt0 first (x2mm0 needs it), then sq2's other half,
            # then the remaining xt squares.
            sqx_s = [sxp.tile([128, 2, KT, 128], FP8, tag="sx", name=f"sx{s}")
                     for s in range(XT_Q)]
            nc.gpsimd.tensor_mul(sqx_s[0], xt_s[0], xt_s[0])
            nc.gpsimd.tensor_mul(sq2[:, 0, :], at_s[2][:, 0, :],
                                 at_s[2][:, 0, :])
            for s in range(1, XT_Q):
                nc.gpsimd.tensor_mul(sqx_s[s], xt_s[s], xt_s[s])

            # --- PE warm-up across the DMA head (p-state ramp)
            warm_ps = psp.tile([128, PSW], F32, tag="ps", name="warm_ps")
            for wi in range(N_WARM):
                nc.tensor.matmul(
                    warm_ps[:64, :512], warm_lhs, warm_src,
                    start=(wi == 0), stop=(wi == N_WARM - 1),
                )

            a2b = big.tile([128, NJ], BF16, tag="a2b")   # -0.5*a2[j] broadcast

            def emit_mains(pts, m):
                for q in range(K2):
                    lhsT = xt_sl(m, q)
                    for n0, w in N_CHUNKS:
                        nc.tensor.matmul(
                            pts[:, n0 : n0 + w], lhsT,
                            at_s[q][:, :, n0 : n0 + w],
                            start=(q == 0), stop=(q == K2 - 1),
                            perf_mode=mybir.MatmulPerfMode.DoubleRow,
                        )

            def emit_x2mm(pts, m):
                sx = sqx_s[m // 2]
                for q in range(K2):
                    nc.tensor.matmul(
                        pts[:, XC : XC + 1],
                        sx[:, m % 2, 2 * q : 2 * q + 2, :], ones2,
                        start=(q == 0), stop=(q == K2 - 1),
                        perf_mode=mybir.MatmulPerfMode.DoubleRow,
                    )

            def emit_a2_pass(ps, q):
                # psa2 += -0.5 * at[k-pair q]^2: DR for the fp8 pairs (0, 2),
                # two plain bf16 passes for pair 1.
                if q == 0:
                    for n0, w in N_CHUNKS:
                        nc.tensor.matmul(
                            ps[:, n0 : n0 + w], neg2, sq0[:, :, n0 : n0 + w],
                            start=True, stop=False,
                            perf_mode=mybir.MatmulPerfMode.DoubleRow,
                        )
                elif q == 1:
                    for r in range(2):
                        for n0, w in N_CHUNKS:
                            nc.tensor.matmul(
                                ps[:, n0 : n0 + w], neg1,
                                sq1[:, r, n0 : n0 + w],
                                start=False, stop=False,
                            )
                else:
                    for n0, w in N_CHUNKS:
                        nc.tensor.matmul(
                            ps[:, n0 : n0 + w], neg2, sq2[:, :, n0 : n0 + w],
                            start=False, stop=True,
                            perf_mode=mybir.MatmulPerfMode.DoubleRow,
                        )

            tcopies = []
            for m in range(MT):
                pts = psp.tile([128, PSW], F32, tag="ps", name=f"ps{m}")
                emit_mains(pts, m)

                if m < 2:
                    # a2 pass first (keeps the PE stream gapless), then the
                    # x2 matmul; psum slot freed early via a copy on idle ACT,
                    # the a2b add lands later into a fresh tile.
                    if m == 0:
                        psa2 = psp.tile([128, PSW], F32, tag="ps", name="psa2")
                        emit_a2_pass(psa2, 0)
                    else:
                        emit_a2_pass(psa2, 1)
                        emit_a2_pass(psa2, 2)
                    emit_x2mm(pts, m)
                    t_e = work.tile([128, NJ], BF16, tag="te", name=f"te{m}",
                                    bufs=2)
                    nc.scalar.copy(t_e, pts[:, :NJ])
                    x2 = work.tile([128, 1], F32, tag="x2", name=f"x2_{m}",
                                   bufs=4)
                    nc.scalar.copy(x2, pts[:, XC : XC + 1])
                    tcopies.append((t_e, x2))
                    if m == 1:
                        nc.vector.tensor_copy(a2b, psa2[:, :NJ])
                    continue

                emit_x2mm(pts, m)
                x2 = work.tile([128, 1], F32, tag="x2", name=f"x2_{m}", bufs=4)
                nc.scalar.copy(x2, pts[:, XC : XC + 1])

                out_t = outs.tile([128, NJ], BF16, tag="out", name=f"out{m}")
                halves = [(0, NJ)] if m < MT - 2 else [(0, NJ // 2), (NJ // 2, NJ)]
                for h0, h1 in halves:
                    t = work.tile([128, NJ], BF16, tag="t", name=f"t{m}_{h0}",
                                  bufs=2)
                    nc.vector.tensor_add(
                        t[:, : h1 - h0], pts[:, h0:h1], a2b[:, h0:h1]
                    )
                    nc.scalar.activation(
                        out_t[:, h0:h1], t[:, : h1 - h0],
                        mybir.ActivationFunctionType.Sqrt,
                        bias=x2, scale=-2.0,
                    )
                    nc.sync.dma_start(
                        out=out_ext[ts(m, 128), h0:h1], in_=out_t[:, h0:h1]
                    )
                if m - 2 < len(tcopies):
                    # finish a deferred early tile behind this tile's add so
                    # the DVE burst never delays a psum-freeing add
                    t_e, x2e = tcopies[m - 2]
                    t_f = work.tile([128, NJ], BF16, tag="t",
                                    name=f"tf{m - 2}", bufs=2)
                    nc.vector.tensor_add(t_f, t_e, a2b)
                    out_t = outs.tile([128, NJ], BF16, tag="out",
                                      name=f"oute{m - 2}")
                    nc.scalar.activation(
                        out_t, t_f, mybir.ActivationFunctionType.Sqrt,
                        bias=x2e, scale=-2.0,
                    )
                    nc.sync.dma_start(
                        out=out_ext[ts(m - 2, 128), :], in_=out_t
                    )

    nc.compile()
    return nc


def make_in_maps(x32: np.ndarray, a32: np.ndarray) -> list[dict[str, np.ndarray]]:
    xt_f8 = x32.T.astype(NP_FP8)           # [E, B]
    at_f8 = a32.T.astype(NP_FP8)           # [E, J]
    in_maps = []
    for c in range(8):
        g, h = c // RJ, c % RJ
        in_maps.append({
            "at": pack_rows(at_f8[:, h * NJ : (h + 1) * NJ]),
            "xt": pack_xt(xt_f8[:, g * MB : (g + 1) * MB]),
        })
    return in_maps


def kernel(x: np.ndarray, anchors: np.ndarray) -> np.ndarray:
    x32 = np.asarray(x, dtype=np.float32)
    a32 = np.asarray(anchors, dtype=np.float32).reshape(J, E)

    nc = build_graph()
    in_maps = make_in_maps(x32, a32)
    results = run_bass_kernel_spmd(nc, in_maps, core_ids=list(range(8))).results

    out = np.empty((B, J), dtype=np.float32)
    for c in range(8):
        g, h = c // RJ, c % RJ
        out[g * MB : (g + 1) * MB, h * NJ : (h + 1) * NJ] = results[c][
            "out"
        ].astype(np.float32)
    return out.reshape(B, C, A)
